# revision 19
# baseline (speedup 1.0000x reference)
"""Trainium2 Bass kernel for the sparse-conv hyper-prior encoder.

Strategy (8-core SPMD, slab-sharded on reconstructed coarse-x):
  host: BFS-reconstruct coarse coords from map3, fine coords from map2;
        slab-shard; build per-shard gather maps; dense zero-embedded feats.
  P1: indirect-gather dense dz-triples (9 descs/point) -> im2col [128,216]
      -> PE transpose -> y1^T = W1all^T @ im2col^T (channel-major)
      -> ACT bias+relu -> Z2 = h1 @ W2all -> DRAM.
  P2: gather 9 Z2-slices/out (incl. bias slot) -> DVE reduce -> relu
      -> transpose -> Z3 = h2 @ W3all -> DRAM (local + boundary windows).
  AllGather boundary-window Z3 -> halo region of Z3tab.
  P3: gather 28 Z3-slices/out (incl. bias slot) -> DVE reduce -> out.
"""
import sys
import numpy as np

if "/opt/trn_rl_repo" not in sys.path:
    sys.path.insert(0, "/opt/trn_rl_repo")

N1 = 300000
C = 64
K27 = 27
K8 = 8
NCORE = 8
P = 128

_r = np.arange(-1, 2)
OFF27 = np.stack(np.meshgrid(_r, _r, _r, indexing='ij'), -1).reshape(-1, 3)
_r2 = np.arange(0, 2)
OFF8 = np.stack(np.meshgrid(_r2, _r2, _r2, indexing='ij'), -1).reshape(-1, 3)


# ---------------------------------------------------------------- host prep
def _reconstruct_coarse_coords(map3_in, map3_out, n2):
    nbr = np.full((K27, n2), -1, np.int64)
    for k in range(K27):
        oi = map3_out[k]; ii = map3_in[k]
        valid = oi != n2
        nbr[k, oi[valid]] = ii[valid]
    coords = np.full((n2, 3), np.iinfo(np.int32).min, np.int64)
    visited = np.zeros(n2, bool)
    x_base = 0
    seeds = np.arange(n2)
    while True:
        unv = seeds[~visited[seeds]]
        if len(unv) == 0:
            break
        s = unv[0]
        coords[s] = (x_base, 0, 0)
        visited[s] = True
        frontier = np.array([s])
        comp_nodes = [frontier]
        while len(frontier):
            new_nodes = []
            for k in range(K27):
                nb = nbr[k, frontier]
                m = nb >= 0
                nb = nb[m]; src = frontier[m]
                m2 = ~visited[nb]
                nb2, idx = np.unique(nb[m2], return_index=True)
                src2 = src[m2][idx]
                if len(nb2):
                    coords[nb2] = coords[src2] + OFF27[k]
                    visited[nb2] = True
                    new_nodes.append(nb2)
            frontier = (np.concatenate(new_nodes) if new_nodes
                        else np.array([], np.int64))
            comp_nodes.append(frontier)
        comp = np.concatenate(comp_nodes)
        cmin = coords[comp].min(0)
        coords[comp] -= cmin
        coords[comp, 0] += x_base
        x_base = coords[comp, 0].max() + 2
    return coords


def _prepare(inputs):
    map2_in = np.asarray(inputs['map2_in']); map2_out = np.asarray(inputs['map2_out'])
    map3_in = np.asarray(inputs['map3_in']); map3_out = np.asarray(inputs['map3_out'])
    n2 = int(inputs['n2'])

    cc = _reconstruct_coarse_coords(map3_in, map3_out, n2)
    fc = np.full((N1, 3), -1, np.int64)
    parent = np.full(N1, -1, np.int64)
    for k in range(K8):
        oi = map2_out[k]; ii = map2_in[k]
        valid = oi != n2
        fc[ii[valid]] = 2 * cc[oi[valid]] + OFF8[k]
        parent[ii[valid]] = oi[valid]
    assert (fc >= 0).all()

    cx = cc[:, 0]
    xmax = int(cx.max())
    counts = np.bincount(cx, minlength=xmax + 1)
    cum = np.cumsum(counts)
    bounds = [0]
    for c in range(1, NCORE):
        bounds.append(int(np.searchsorted(cum, c * n2 / NCORE) + 1))
    bounds.append(xmax + 1)
    slab_of_plane = np.zeros(xmax + 1, np.int64)
    for c in range(NCORE):
        slab_of_plane[bounds[c]:bounds[c + 1]] = c
    slab = slab_of_plane[cx]

    Wp = 0
    for c in range(NCORE):
        lo, hi = bounds[c], bounds[c + 1] - 1
        Wp = max(Wp, int((cx == lo).sum()), int((cx == hi).sum()))
    Wp = ((Wp + P - 1) // P) * P

    loc = []
    for c in range(NCORE):
        lo, hi = bounds[c], bounds[c + 1] - 1
        ids_lo = np.where(cx == lo)[0]
        if hi != lo:
            ids_hi = np.where((cx == hi) & (slab == c))[0]
            ids_int = np.where((slab == c) & (cx != lo) & (cx != hi))[0]
        else:
            ids_hi = np.array([], np.int64)
            ids_int = np.array([], np.int64)
        loc.append((ids_lo, ids_hi, ids_int))

    maxint = max(len(t[2]) for t in loc)
    Lc = 2 * Wp + maxint
    T2 = (Lc + P - 1) // P
    Lc = T2 * P

    coarse_local = np.full((NCORE, Lc), -1, np.int64)
    coarse_g2l = np.full(n2, -1, np.int64)
    for c in range(NCORE):
        ids_lo, ids_hi, ids_int = loc[c]
        coarse_local[c, :len(ids_lo)] = ids_lo
        coarse_local[c, Wp:Wp + len(ids_hi)] = ids_hi
        coarse_local[c, 2 * Wp:2 * Wp + len(ids_int)] = ids_int
        coarse_g2l[ids_lo] = np.arange(len(ids_lo))
        coarse_g2l[ids_hi] = Wp + np.arange(len(ids_hi))
        coarse_g2l[ids_int] = 2 * Wp + np.arange(len(ids_int))

    fslab = slab[parent]
    fine_ids = [np.where(fslab == c)[0] for c in range(NCORE)]
    maxfine = max(len(a) for a in fine_ids)
    T1 = (maxfine + P - 1) // P
    Lf = T1 * P
    fine_local = np.full((NCORE, Lf), -1, np.int64)
    fine_g2l = np.full(N1, -1, np.int64)
    for c in range(NCORE):
        a = fine_ids[c]
        fine_local[c, :len(a)] = a
        fine_g2l[a] = np.arange(len(a))

    fxmax = int(fc[:, 0].max())
    DX, DY, DZ = fxmax + 3, 130, 130
    dense_id = ((fc[:, 0] + 1) * DY + (fc[:, 1] + 1)) * DZ + (fc[:, 2] + 1)
    Vdf = DX * DY * DZ

    off9 = OFF27.reshape(9, 3, 3)[:, 0, :2]
    G1 = np.zeros((NCORE, Lf, 9), np.int64)
    for c in range(NCORE):
        ids = fine_local[c]
        m = ids >= 0
        base = np.zeros((Lf, 3), np.int64)
        base[m] = fc[ids[m]]
        for g in range(9):
            dx, dy = off9[g]
            did = ((base[:, 0] + 1 + dx) * DY +
                   (base[:, 1] + 1 + dy)) * DZ + (base[:, 2] + 1 - 1)
            did[~m] = 0
            G1[c, :, g] = did
    assert G1.max() < Vdf and G1.min() >= 0

    Z2_rows = Lf * 8 + 2
    Z2_zero = Lf * 8
    Z2_bias = Lf * 8 + 1
    G2 = np.full((NCORE, Lc, K8 + 1), Z2_zero, np.int64)
    G2[:, :, K8] = Z2_bias
    for k in range(K8):
        oi = map2_out[k]; ii = map2_in[k]
        valid = oi != n2
        o = oi[valid]; i_ = ii[valid]
        c_of = slab[o]
        for c in range(NCORE):
            m = c_of == c
            G2[c, coarse_g2l[o[m]], k] = fine_g2l[i_[m]] * 8 + k
    assert G2.max() < Z2_rows

    halo_base = Lc
    Z3_rows = (Lc + NCORE * 2 * Wp) * K27 + 32
    Z3_zero = (Lc + NCORE * 2 * Wp) * K27
    Z3_bias = Z3_zero + 1
    G3 = np.full((NCORE, Lc, K27 + 1), Z3_zero, np.int64)
    G3[:, :, K27] = Z3_bias
    for k in range(K27):
        oi = map3_out[k]; ii = map3_in[k]
        valid = oi != n2
        o = oi[valid]; i_ = ii[valid]
        c_of = slab[o]
        i_slab = slab[i_]
        for c in range(NCORE):
            m = c_of == c
            om = o[m]; im = i_[m]
            own = i_slab[m] == c
            G3[c, coarse_g2l[om[own]], k] = coarse_g2l[im[own]] * K27 + k
            d = i_slab[m][~own]
            hp = coarse_g2l[im[~own]]
            assert (hp < 2 * Wp).all()
            G3[c, coarse_g2l[om[~own]], k] = (halo_base + d * 2 * Wp + hp) * K27 + k
    assert G3.max() < Z3_rows

    W1 = np.asarray(inputs['W1'], np.float32)
    W2 = np.asarray(inputs['W2'], np.float32)
    W3 = np.asarray(inputs['W3'], np.float32)
    W1all = np.zeros((216, C), np.float32)
    for g in range(9):
        for dzi in range(3):
            k = g * 3 + dzi
            W1all[k * 8:(k + 1) * 8, :] = W1[k]
    W2all = np.zeros((C, 8 * C), np.float32)
    for j in range(K8):
        W2all[:, j * C:(j + 1) * C] = W2[j]
    W3all = np.zeros((C, K27 * 8), np.float32)
    for k in range(K27):
        W3all[:, k * 8:(k + 1) * 8] = W3[k]

    return dict(
        n2=n2, Wp=Wp, Lc=Lc, Lf=Lf, T1=T1, T2=T2, Vdf=Vdf,
        dense_id=dense_id, coarse_local=coarse_local, fine_local=fine_local,
        G1=G1, G2=G2, G3=G3, W1all=W1all, W2all=W2all, W3all=W3all,
        Z2_rows=Z2_rows, Z3_rows=Z3_rows, Z3_zero=Z3_zero,
        b1=np.asarray(inputs['b1'], np.float32),
        b2=np.asarray(inputs['b2'], np.float32),
        b3=np.asarray(inputs['b3'], np.float32),
    )


# ---------------------------------------------------------------- device
def _build(info, TB1=16, TB2=8, TB3=16):
    from contextlib import ExitStack
    import concourse.bass as bass
    import concourse.mybir as mybir
    from concourse import tile
    dt = mybir.dt

    T1, T2, Wp, Lc, Lf = info['T1'], info['T2'], info['Wp'], info['Lc'], info['Lf']
    Vdf = info['Vdf']
    Z2_rows, Z3_rows, Z3_zero = info['Z2_rows'], info['Z3_rows'], info['Z3_zero']

    nc = bass.Bass()

    fd = nc.declare_dram_parameter("fd", [Vdf, 8], dt.float32, isOutput=False)
    g1d = nc.declare_dram_parameter("g1", [P, T1 * 9], dt.int32, isOutput=False)
    g2d = nc.declare_dram_parameter("g2", [P, T2 * 9], dt.int32, isOutput=False)
    g3d = nc.declare_dram_parameter("g3", [P, T2 * 28], dt.int32, isOutput=False)
    w1ad = nc.declare_dram_parameter("w1a", [128, 64], dt.bfloat16, isOutput=False)
    w1bd = nc.declare_dram_parameter("w1b", [128, 64], dt.bfloat16, isOutput=False)
    w2d = nc.declare_dram_parameter("w2", [64, 512], dt.bfloat16, isOutput=False)
    w3d = nc.declare_dram_parameter("w3", [64, 216], dt.bfloat16, isOutput=False)
    b1d = nc.declare_dram_parameter("b1v", [64, 1], dt.float32, isOutput=False)
    zb2d = nc.declare_dram_parameter("zb2", [2, 64], dt.bfloat16, isOutput=False)
    zb3d = nc.declare_dram_parameter("zb3", [32, 8], dt.bfloat16, isOutput=False)
    yout = nc.declare_dram_parameter("yout", [Lc, 8], dt.float32, isOutput=True)
    import os as _os
    _dbg = _os.environ.get("KDBG", "0") == "1"
    if _dbg:
        dbgZ2 = nc.declare_dram_parameter("dbgZ2", [4096, 64], dt.bfloat16, isOutput=True)
        dbgZ3 = nc.declare_dram_parameter("dbgZ3", [4096, 8], dt.bfloat16, isOutput=True)
        dbgH = nc.declare_dram_parameter("dbgH", [4096, 8], dt.bfloat16, isOutput=True)

    Z2 = nc.dram_tensor("Z2", [Z2_rows, 64], dt.bfloat16)
    Z3tab = nc.dram_tensor("Z3tab", [Z3_rows, 8], dt.bfloat16)
    Z3win = nc.dram_tensor("Z3win", [2 * Wp * K27, 8], dt.bfloat16)
    Z3halo = nc.dram_tensor("Z3halo", [NCORE * 2 * Wp * K27, 8], dt.bfloat16,
                            addr_space="Shared")

    # ---------------- raw prelude: constants + tail rows ----------------
    ctx = ExitStack()
    g1sb = ctx.enter_context(nc.sbuf_tensor([P, T1 * 9], dt.int32))
    g2sb = ctx.enter_context(nc.sbuf_tensor([P, T2 * 9], dt.int32))
    g3sb = ctx.enter_context(nc.sbuf_tensor([P, T2 * 28], dt.int32))
    w1a = ctx.enter_context(nc.sbuf_tensor([128, 64], dt.bfloat16))
    w1b = ctx.enter_context(nc.sbuf_tensor([128, 64], dt.bfloat16))
    w2 = ctx.enter_context(nc.sbuf_tensor([64, 512], dt.bfloat16))
    w3 = ctx.enter_context(nc.sbuf_tensor([64, 216], dt.bfloat16))
    b1sb = ctx.enter_context(nc.sbuf_tensor([64, 1], dt.float32))
    zb2 = ctx.enter_context(nc.sbuf_tensor([2, 64], dt.bfloat16))
    zb3 = ctx.enter_context(nc.sbuf_tensor([32, 8], dt.bfloat16))
    ident = ctx.enter_context(nc.sbuf_tensor([P, P], dt.float32))

    # work buffers (double/triple buffered)
    NIM, NZ2 = 3, 3
    imb = [ctx.enter_context(nc.sbuf_tensor(f"imb{i}", [P, TB1 * 216], dt.float32))
           for i in range(NIM)]
    r1b = [ctx.enter_context(nc.sbuf_tensor(f"r1b{i}", [128, 128], dt.bfloat16)) for i in range(2)]
    r2b = [ctx.enter_context(nc.sbuf_tensor(f"r2b{i}", [128, 128], dt.bfloat16)) for i in range(2)]
    h1Tb = [ctx.enter_context(nc.sbuf_tensor(f"h1Tb{i}", [64, 128], dt.bfloat16)) for i in range(2)]
    z2b = [ctx.enter_context(nc.sbuf_tensor(f"z2b{i}", [128, 512], dt.bfloat16))
           for i in range(NZ2)]
    d2b = [ctx.enter_context(nc.sbuf_tensor(f"d2b{i}", [P, TB2 * 576], dt.bfloat16))
           for i in range(2)]
    redb = [ctx.enter_context(nc.sbuf_tensor(f"redb{i}", [128, 64], dt.float32)) for i in range(2)]
    h2b = [ctx.enter_context(nc.sbuf_tensor(f"h2b{i}", [128, 64], dt.float32)) for i in range(2)]
    h2Tb = [ctx.enter_context(nc.sbuf_tensor(f"h2Tb{i}", [64, 128], dt.bfloat16)) for i in range(2)]
    z3b = [ctx.enter_context(nc.sbuf_tensor(f"z3b{i}", [128, 216], dt.bfloat16))
           for i in range(NZ2)]
    d3b = [ctx.enter_context(nc.sbuf_tensor(f"d3b{i}", [P, TB3 * 224], dt.bfloat16))
           for i in range(2)]
    ob = [ctx.enter_context(nc.sbuf_tensor(f"ob{i}", [128, 8], dt.float32)) for i in range(3)]

    pt1b = [ctx.enter_context(nc.psum_tensor(f"pt1b{i}", [128, 128], dt.float32))
            for i in range(2)]
    pt2b = [ctx.enter_context(nc.psum_tensor(f"pt2b{i}", [128, 128], dt.float32))
            for i in range(2)]
    pyb = [ctx.enter_context(nc.psum_tensor(f"pyb{i}", [64, 128], dt.float32)) for i in range(2)]
    pzb = [ctx.enter_context(nc.psum_tensor(f"pzb{i}", [128, 512], dt.float32)) for i in range(2)]

    sem = {}
    names = ["s_ld", "s_id", "sVr", "sTt", "sVcp", "sTmm", "sAr", "sTz",
             "sVz", "sVh2", "sTt2", "sVhT", "sTz3", "sVz3",
             "sVo", "sh", "scc"]
    names += [f"sGim{i}" for i in range(NIM)]
    names += [f"sS{i}" for i in range(NZ2)]
    names += ["sGd20", "sGd21"]
    names += [f"sS2_{i}" for i in range(NZ2)]
    names += ["sGd30", "sGd31"]
    names += [f"sS3_{i}" for i in range(3)]
    for name in names:
        sem[name] = ctx.enter_context(nc.semaphore(name))

    loads = [(g1sb, g1d), (g2sb, g2d), (g3sb, g3d), (w1a, w1ad), (w1b, w1bd),
             (w2, w2d), (w3, w3d), (b1sb, b1d), (zb2, zb2d), (zb3, zb3d)]
    for dst, src in loads:
        nc.sync.dma_start(out=dst[:], in_=src[:]).then_inc(sem["s_ld"], 16)
    nc.sync.wait_ge(sem["s_ld"], 16 * len(loads))
    nc.sync.dma_start(out=Z2[Lf * 8:Lf * 8 + 2, :], in_=zb2[:]).then_inc(sem["s_ld"], 16)
    nc.sync.dma_start(out=Z3tab[Z3_zero:Z3_zero + 32, :], in_=zb3[:]).then_inc(
        sem["s_ld"], 16)
    nc.gpsimd.memset(ident[:], 0.0).then_inc(sem["s_id"], 1)
    nc.gpsimd.wait_ge(sem["s_id"], 1)
    nc.gpsimd.affine_select(
        out=ident[:], in_=ident[:],
        compare_op=mybir.AluOpType.not_equal, fill=1.0, base=0,
        pattern=[[-1, P]], channel_multiplier=1,
    ).then_inc(sem["s_id"], 1)
    nc.gpsimd.memset(r2b[0][64:128, :], 0.0).then_inc(sem["s_id"], 1)
    nc.gpsimd.memset(r2b[1][64:128, :], 0.0).then_inc(sem["s_id"], 1)
    nwait = 16 * (len(loads) + 2)
    for eng in (nc.sync, nc.gpsimd, nc.tensor, nc.vector, nc.scalar):
        eng.wait_ge(sem["s_ld"], nwait)
        eng.wait_ge(sem["s_id"], 4)

    def W(eng, s, v):
        if v > 0:
            eng.wait_ge(sem[s], v)

    # ================= phase 1 =================
    NB1 = (T1 + TB1 - 1) // TB1
    batch_of = lambda t: t // TB1
    end_tile = lambda b: min((b + 1) * TB1, T1)

    # gpsimd: all gather batches ([128,1]-offset calls; multi-idx broken on HW)
    gim_cnt = [0] * NIM
    gim_at = {}
    for b in range(NB1):
        nt = min(TB1, T1 - b * TB1)
        if b >= NIM:
            W(nc.gpsimd, "sTt", end_tile(b - NIM))
        sl = b % NIM
        for j in range(nt * 9):
            nc.gpsimd.indirect_dma_start(
                out=imb[sl][:, j * 24:(j + 1) * 24], out_offset=None, in_=fd[:],
                in_offset=bass.IndirectOffsetOnAxis(
                    ap=g1sb[:, b * TB1 * 9 + j:b * TB1 * 9 + j + 1], axis=0),
            ).then_inc(sem[f"sGim{sl}"], 16)
            gim_cnt[sl] += 16
        gim_at[b] = gim_cnt[sl]

    for t in range(T1):
        b = batch_of(t)
        tt = t - b * TB1
        s_ap = imb[b % NIM][:, tt * 216:(tt + 1) * 216]
        i2 = t % 2

        # --- tensor engine ---
        if tt == 0:
            W(nc.tensor, f"sGim{b % NIM}", gim_at[b])
        W(nc.tensor, "sVcp", 2 * (t - 2) + 2)
        nc.tensor.transpose(out=pt1b[i2][:], in_=s_ap[:, 0:128], identity=ident[:])
        nc.tensor.transpose(out=pt2b[i2][:88, :], in_=s_ap[:, 128:216],
                            identity=ident[:]).then_inc(sem["sTt"], 1)
        W(nc.tensor, "sVcp", 2 * t + 2)
        W(nc.tensor, "sAr", t - 1)
        nc.tensor.matmul(out=pyb[i2][:], lhsT=w1a[:], rhs=r1b[i2][:],
                         start=True, stop=False)
        nc.tensor.matmul(out=pyb[i2][:], lhsT=w1b[:], rhs=r2b[i2][:],
                         start=False, stop=True).then_inc(sem["sTmm"], 1)
        W(nc.tensor, "sAr", t + 1)
        W(nc.tensor, "sVz", t - 1)
        nc.tensor.matmul(out=pzb[i2][:], lhsT=h1Tb[i2][:], rhs=w2[:],
                         start=True, stop=True).then_inc(sem["sTz"], 1)

        # --- vector engine ---
        W(nc.vector, "sTt", t + 1)
        W(nc.vector, "sTmm", t - 1)
        nc.vector.tensor_copy(out=r1b[i2][:], in_=pt1b[i2][:])
        nc.vector.tensor_copy(out=r2b[i2][:88, :], in_=pt2b[i2][:88, :]).then_inc(
            sem["sVcp"], 2)
        if t >= 1:
            W(nc.vector, "sTz", t)
            W(nc.vector, f"sS{(t - 1) % NZ2}", 16 * ((t - 1) // NZ2))
            nc.vector.tensor_copy(out=z2b[(t - 1) % NZ2][:],
                                  in_=pzb[(t - 1) % 2][:]).then_inc(sem["sVz"], 1)

        # --- scalar engine ---
        W(nc.scalar, "sTmm", t + 1)
        W(nc.scalar, "sTz", t - 1)
        nc.scalar.activation(out=h1Tb[i2][:], in_=pyb[i2][:],
                             func=mybir.ActivationFunctionType.Relu,
                             bias=b1sb[:, 0:1]).then_inc(sem["sAr"], 1)

        # --- sync engine: write z2 of tile t-1 ---
        if t >= 1:
            W(nc.sync, "sVz", t)
            nc.sync.dma_start(
                out=Z2[(t - 1) * 1024:t * 1024, :].rearrange(
                    "(p j) c -> p (j c)", p=128),
                in_=z2b[(t - 1) % NZ2][:]).then_inc(sem[f"sS{(t - 1) % NZ2}"], 16)
    # tail
    W(nc.vector, "sTz", T1)
    W(nc.vector, f"sS{(T1 - 1) % NZ2}", 16 * ((T1 - 1) // NZ2))
    nc.vector.tensor_copy(out=z2b[(T1 - 1) % NZ2][:],
                          in_=pzb[(T1 - 1) % 2][:]).then_inc(sem["sVz"], 1)
    W(nc.sync, "sVz", T1)
    nc.sync.dma_start(
        out=Z2[(T1 - 1) * 1024:T1 * 1024, :].rearrange("(p j) c -> p (j c)", p=128),
        in_=z2b[(T1 - 1) % NZ2][:]).then_inc(sem[f"sS{(T1 - 1) % NZ2}"], 16)
    # phase-1 completion barrier: all engines wait for all Z2 writes
    for eng in (nc.sync, nc.gpsimd, nc.tensor, nc.vector, nc.scalar):
        for k in range(NZ2):
            cnt = len([t for t in range(T1) if t % NZ2 == k])
            eng.wait_ge(sem[f"sS{k}"], 16 * cnt)

    # ================= phase 2 =================
    NB2 = (T2 + TB2 - 1) // TB2
    end_tile2 = lambda b: min((b + 1) * TB2, T2)
    gd2_cnt = [0, 0]
    gd2_at = {}
    for b in range(NB2):
        nt = min(TB2, T2 - b * TB2)
        if b >= 2:
            W(nc.gpsimd, "sVh2", end_tile2(b - 2))
        sl = b % 2
        for j in range(nt * 9):
            nc.gpsimd.indirect_dma_start(
                out=d2b[sl][:, j * 64:(j + 1) * 64], out_offset=None, in_=Z2[:],
                in_offset=bass.IndirectOffsetOnAxis(
                    ap=g2sb[:, b * TB2 * 9 + j:b * TB2 * 9 + j + 1], axis=0),
            ).then_inc(sem[f"sGd2{sl}"], 16)
            gd2_cnt[sl] += 16
        gd2_at[b] = gd2_cnt[sl]

    for t in range(T2):
        b = t // TB2
        tt = t - b * TB2
        s_ap = d2b[b % 2][:, tt * 576:(tt + 1) * 576]
        i2 = t % 2

        # --- vector: reduce + relu ---
        if tt == 0:
            W(nc.vector, f"sGd2{b % 2}", gd2_at[b])
        W(nc.vector, "sTt2", t - 1)
        nc.vector.tensor_reduce(
            out=redb[i2][:], in_=s_ap.rearrange("p (j c) -> p c j", j=9),
            axis=mybir.AxisListType.X, op=mybir.AluOpType.add).then_inc(
            sem["sVr"], 1)
        W(nc.vector, "sVr", t + 1)
        nc.vector.tensor_scalar_max(out=h2b[i2][:], in0=redb[i2][:],
                                    scalar1=0.0).then_inc(sem["sVh2"], 1)
        if t >= 1:
            W(nc.vector, "sTt2", t)
            W(nc.vector, "sTz3", t - 2)
            nc.vector.tensor_copy(out=h2Tb[(t - 1) % 2][:],
                                  in_=pt1b[(t - 1) % 2][:64, :]).then_inc(
                sem["sVhT"], 1)
            W(nc.vector, "sTz3", t)
            W(nc.vector, f"sS2_{(t - 1) % NZ2}", 16 * ((t - 1) // NZ2))
            nc.vector.tensor_copy(out=z3b[(t - 1) % NZ2][:],
                                  in_=pzb[(t - 1) % 2][:, 0:216]).then_inc(
                sem["sVz3"], 1)

        # --- tensor: transpose(t), then matmul(t-1) ---
        W(nc.tensor, "sVh2", t + 1)
        W(nc.tensor, "sVhT", t - 1)
        nc.tensor.transpose(out=pt1b[i2][:64, :], in_=h2b[i2][:],
                            identity=ident[:]).then_inc(sem["sTt2"], 1)
        if t >= 1:
            W(nc.tensor, "sVhT", t)
            W(nc.tensor, "sVz3", t - 2)
            nc.tensor.matmul(out=pzb[(t - 1) % 2][:, 0:216],
                             lhsT=h2Tb[(t - 1) % 2][:], rhs=w3[:],
                             start=True, stop=True).then_inc(sem["sTz3"], 1)

        # --- sync: write z3 of t-2 ---
        if t >= 2:
            W(nc.sync, "sVz3", t - 1)
            nc.sync.dma_start(
                out=Z3tab[(t - 2) * 3456:(t - 1) * 3456, :].rearrange(
                    "(p k) c -> p (k c)", p=128),
                in_=z3b[(t - 2) % NZ2][:]).then_inc(sem[f"sS2_{(t - 2) % NZ2}"], 16)
    # tail of phase 2
    t = T2
    W(nc.vector, "sTt2", t)
    nc.vector.tensor_copy(out=h2Tb[(t - 1) % 2][:],
                          in_=pt1b[(t - 1) % 2][:64, :]).then_inc(sem["sVhT"], 1)
    W(nc.tensor, "sVhT", t)
    nc.tensor.matmul(out=pzb[(t - 1) % 2][:, 0:216], lhsT=h2Tb[(t - 1) % 2][:],
                     rhs=w3[:], start=True, stop=True).then_inc(sem["sTz3"], 1)
    W(nc.vector, "sTz3", t)
    W(nc.vector, f"sS2_{(t - 1) % NZ2}", 16 * ((t - 1) // NZ2))
    nc.vector.tensor_copy(out=z3b[(t - 1) % NZ2][:],
                          in_=pzb[(t - 1) % 2][:, 0:216]).then_inc(sem["sVz3"], 1)
    for tl in (T2 - 1, T2):
        W(nc.sync, "sVz3", tl)
        nc.sync.dma_start(
            out=Z3tab[(tl - 1) * 3456:tl * 3456, :].rearrange(
                "(p k) c -> p (k c)", p=128),
            in_=z3b[(tl - 1) % NZ2][:]).then_inc(sem[f"sS2_{(tl - 1) % NZ2}"], 16)
    for eng in (nc.sync, nc.gpsimd, nc.tensor, nc.vector, nc.scalar):
        for k in range(NZ2):
            cnt = len([t for t in range(T2) if t % NZ2 == k])
            eng.wait_ge(sem[f"sS2_{k}"], 16 * cnt)

    # ================= halo exchange =================
    nc.sync.dma_start(out=Z3win[:], in_=Z3tab[0:2 * Wp * K27, :]).then_inc(
        sem["sh"], 16)
    nc.gpsimd.wait_ge(sem["sh"], 16)
    nc.gpsimd.collective_compute(
        "AllGather", mybir.AluOpType.bypass,
        replica_groups=[list(range(NCORE))],
        ins=[Z3win[:]], outs=[Z3halo[:]],
    ).then_inc(sem["scc"], 1)
    nc.sync.wait_ge(sem["scc"], 1)
    nc.sync.dma_start(
        out=Z3tab[Lc * K27:Lc * K27 + NCORE * 2 * Wp * K27, :],
        in_=Z3halo[:]).then_inc(sem["sh"], 16)
    nc.gpsimd.wait_ge(sem["sh"], 32)
    nc.vector.wait_ge(sem["sh"], 32)

    # ================= phase 3 =================
    NB3 = (T2 + TB3 - 1) // TB3
    end_tile3 = lambda b: min((b + 1) * TB3, T2)
    gd3_cnt = [0, 0]
    gd3_at = {}
    for b in range(NB3):
        nt = min(TB3, T2 - b * TB3)
        if b >= 2:
            W(nc.gpsimd, "sVo", end_tile3(b - 2))
        sl = b % 2
        for j in range(nt * 28):
            nc.gpsimd.indirect_dma_start(
                out=d3b[sl][:, j * 8:(j + 1) * 8], out_offset=None, in_=Z3tab[:],
                in_offset=bass.IndirectOffsetOnAxis(
                    ap=g3sb[:, b * TB3 * 28 + j:b * TB3 * 28 + j + 1], axis=0),
            ).then_inc(sem[f"sGd3{sl}"], 16)
            gd3_cnt[sl] += 16
        gd3_at[b] = gd3_cnt[sl]
    for t in range(T2):
        b = t // TB3
        tt = t - b * TB3
        s_ap = d3b[b % 2][:, tt * 224:(tt + 1) * 224]
        if tt == 0:
            W(nc.vector, f"sGd3{b % 2}", gd3_at[b])
        W(nc.vector, f"sS3_{t % 3}", 16 * (t // 3))
        nc.vector.tensor_reduce(
            out=ob[t % 3][:], in_=s_ap.rearrange("p (j c) -> p c j", j=28),
            axis=mybir.AxisListType.X, op=mybir.AluOpType.add).then_inc(
            sem["sVo"], 1)
        W(nc.sync, "sVo", t + 1)
        nc.sync.dma_start(out=yout[t * 128:(t + 1) * 128, :],
                          in_=ob[t % 3][:]).then_inc(sem[f"sS3_{t % 3}"], 16)
    for k in range(3):
        cnt = len([t for t in range(T2) if t % 3 == k])
        nc.sync.wait_ge(sem[f"sS3_{k}"], 16 * cnt)
    if _dbg:
        nc.sync.dma_start(out=dbgZ2[:], in_=Z2[0:4096, :]).then_inc(sem["sh"], 16)
        nc.sync.dma_start(out=dbgZ3[:], in_=Z3tab[0:4096, :]).then_inc(sem["sh"], 16)
        nc.sync.dma_start(out=dbgH[:],
                          in_=Z3tab[Lc * K27:Lc * K27 + 4096, :]).then_inc(sem["sh"], 16)
        nc.sync.wait_ge(sem["sh"], 80)

    ctx.close()
    return nc


# ---------------------------------------------------------------- entry
_GRAPH_CACHE = {}


def _ensure_trace_shim():
    """bass_utils imports antenv.axon_hooks when BASS_TRACE is set; the agent
    image lacks that module. Provide it (with the ctypes NTFF hook if the
    axon boot files are present)."""
    import os, types
    if "antenv.axon_hooks" in sys.modules or not os.environ.get("BASS_TRACE"):
        return
    try:
        import antenv
        mod = types.ModuleType("antenv.axon_hooks")
        state = {"hook": None}
        mod.set_axon_ntff_profile_hook = lambda h: state.__setitem__("hook", h)
        mod.get_axon_ntff_profile_hook = lambda: state["hook"]
        sys.modules["antenv.axon_hooks"] = mod
        antenv.axon_hooks = mod
        sys.path.insert(0, "/root/.axon_site")
        from trn_agent_boot.trn_boot import _ntff_profile_via_ctypes
        mod.set_axon_ntff_profile_hook(
            _ntff_profile_via_ctypes("/opt/axon/libaxon_pjrt.so"))
    except Exception:
        pass


def kernel(feats, W1, b1, W2, b2, W3, b3,
           map1_in, map1_out, map2_in, map2_out, map3_in, map3_out, n2):
    import ml_dtypes
    _ensure_trace_shim()
    from concourse.bass_utils import run_bass_kernel_spmd
    bf16 = ml_dtypes.bfloat16

    inputs = dict(feats=np.asarray(feats), W1=W1, b1=b1, W2=W2, b2=b2,
                  W3=W3, b3=b3, map2_in=np.asarray(map2_in),
                  map2_out=np.asarray(map2_out), map3_in=np.asarray(map3_in),
                  map3_out=np.asarray(map3_out), n2=int(n2))
    info = _prepare(inputs)
    T1, T2, Lc, Lf = info['T1'], info['T2'], info['Lc'], info['Lf']

    feats_f = np.asarray(feats, np.float32)
    fdense = np.zeros((info['Vdf'], 8), np.float32)
    fdense[info['dense_id']] = feats_f

    w1a = np.zeros((128, 64), np.float32)
    w1b = np.zeros((128, 64), np.float32)
    w1a[:] = info['W1all'][0:128]
    w1b[:88] = info['W1all'][128:216]
    zb2 = np.zeros((2, 64), bf16)
    zb2[1] = info['b2'].astype(bf16)
    zb3 = np.zeros((32, 8), bf16)
    zb3[1] = info['b3'].astype(bf16)

    shared = dict(
        fd=fdense,
        w1a=w1a.astype(bf16), w1b=w1b.astype(bf16),
        w2=info['W2all'].astype(bf16), w3=info['W3all'].astype(bf16),
        b1v=info['b1'].reshape(64, 1).astype(np.float32),
        zb2=zb2, zb3=zb3,
    )
    in_maps = []
    for c in range(NCORE):
        m = dict(shared)
        m['g1'] = np.ascontiguousarray(
            info['G1'][c].reshape(T1, P, 9).transpose(1, 0, 2)
            .reshape(P, T1 * 9).astype(np.int32))
        m['g2'] = np.ascontiguousarray(
            info['G2'][c].reshape(T2, P, 9).transpose(1, 0, 2)
            .reshape(P, T2 * 9).astype(np.int32))
        m['g3'] = np.ascontiguousarray(
            info['G3'][c].reshape(T2, P, 28).transpose(1, 0, 2)
            .reshape(P, T2 * 28).astype(np.int32))
        in_maps.append(m)

    key = (T1, T2, info['Wp'], info['Vdf'])
    if key not in _GRAPH_CACHE:
        _GRAPH_CACHE[key] = _build(info)
    nc = _GRAPH_CACHE[key]

    res = run_bass_kernel_spmd(nc, in_maps, list(range(NCORE)))
    kernel.last_hw_ns = res.exec_time_ns or 0

    out = np.zeros((info['n2'], 8), np.float32)
    for c in range(NCORE):
        ids = info['coarse_local'][c]
        mm = ids >= 0
        yc = np.asarray(res.results[c]['yout'], np.float32)
        out[ids[mm]] = yc[mm]
    return out


# revision 20
# speedup vs baseline: 1.2144x; 1.2144x over previous
"""Trainium2 Bass kernel for the sparse-conv hyper-prior encoder.

Strategy (8-core SPMD, slab-sharded on reconstructed coarse-x):
  host: BFS-reconstruct coarse coords from map3, fine coords from map2;
        slab-shard; build per-shard gather maps; dense zero-embedded feats.
  P1: indirect-gather dense dz-triples (9 descs/point) -> im2col [128,216]
      -> PE transpose -> y1^T = W1all^T @ im2col^T (channel-major)
      -> ACT bias+relu -> Z2 = h1 @ W2all -> DRAM.
  P2: gather 9 Z2-slices/out (incl. bias slot) -> DVE reduce -> relu
      -> transpose -> Z3 = h2 @ W3all -> DRAM (local + boundary windows).
  AllGather boundary-window Z3 -> halo region of Z3tab.
  P3: gather 28 Z3-slices/out (incl. bias slot) -> DVE reduce -> out.
"""
import sys
import numpy as np

if "/opt/trn_rl_repo" not in sys.path:
    sys.path.insert(0, "/opt/trn_rl_repo")

N1 = 300000
C = 64
K27 = 27
K8 = 8
NCORE = 8
P = 128

_r = np.arange(-1, 2)
OFF27 = np.stack(np.meshgrid(_r, _r, _r, indexing='ij'), -1).reshape(-1, 3)
_r2 = np.arange(0, 2)
OFF8 = np.stack(np.meshgrid(_r2, _r2, _r2, indexing='ij'), -1).reshape(-1, 3)


# ---------------------------------------------------------------- host prep
def _reconstruct_coarse_coords(map3_in, map3_out, n2):
    nbr = np.full((K27, n2), -1, np.int64)
    for k in range(K27):
        oi = map3_out[k]; ii = map3_in[k]
        valid = oi != n2
        nbr[k, oi[valid]] = ii[valid]
    coords = np.full((n2, 3), np.iinfo(np.int32).min, np.int64)
    visited = np.zeros(n2, bool)
    x_base = 0
    seeds = np.arange(n2)
    while True:
        unv = seeds[~visited[seeds]]
        if len(unv) == 0:
            break
        s = unv[0]
        coords[s] = (x_base, 0, 0)
        visited[s] = True
        frontier = np.array([s])
        comp_nodes = [frontier]
        while len(frontier):
            new_nodes = []
            for k in range(K27):
                nb = nbr[k, frontier]
                m = nb >= 0
                nb = nb[m]; src = frontier[m]
                m2 = ~visited[nb]
                nb2, idx = np.unique(nb[m2], return_index=True)
                src2 = src[m2][idx]
                if len(nb2):
                    coords[nb2] = coords[src2] + OFF27[k]
                    visited[nb2] = True
                    new_nodes.append(nb2)
            frontier = (np.concatenate(new_nodes) if new_nodes
                        else np.array([], np.int64))
            comp_nodes.append(frontier)
        comp = np.concatenate(comp_nodes)
        cmin = coords[comp].min(0)
        coords[comp] -= cmin
        coords[comp, 0] += x_base
        x_base = coords[comp, 0].max() + 2
    return coords


def _prepare(inputs):
    map2_in = np.asarray(inputs['map2_in']); map2_out = np.asarray(inputs['map2_out'])
    map3_in = np.asarray(inputs['map3_in']); map3_out = np.asarray(inputs['map3_out'])
    n2 = int(inputs['n2'])

    cc = _reconstruct_coarse_coords(map3_in, map3_out, n2)
    fc = np.full((N1, 3), -1, np.int64)
    parent = np.full(N1, -1, np.int64)
    for k in range(K8):
        oi = map2_out[k]; ii = map2_in[k]
        valid = oi != n2
        fc[ii[valid]] = 2 * cc[oi[valid]] + OFF8[k]
        parent[ii[valid]] = oi[valid]
    assert (fc >= 0).all()

    cx = cc[:, 0]
    xmax = int(cx.max())
    counts = np.bincount(cx, minlength=xmax + 1)
    cum = np.cumsum(counts)
    bounds = [0]
    for c in range(1, NCORE):
        bounds.append(int(np.searchsorted(cum, c * n2 / NCORE) + 1))
    bounds.append(xmax + 1)
    slab_of_plane = np.zeros(xmax + 1, np.int64)
    for c in range(NCORE):
        slab_of_plane[bounds[c]:bounds[c + 1]] = c
    slab = slab_of_plane[cx]

    Wp = 0
    for c in range(NCORE):
        lo, hi = bounds[c], bounds[c + 1] - 1
        Wp = max(Wp, int((cx == lo).sum()), int((cx == hi).sum()))
    Wp = ((Wp + P - 1) // P) * P

    loc = []
    for c in range(NCORE):
        lo, hi = bounds[c], bounds[c + 1] - 1
        ids_lo = np.where(cx == lo)[0]
        if hi != lo:
            ids_hi = np.where((cx == hi) & (slab == c))[0]
            ids_int = np.where((slab == c) & (cx != lo) & (cx != hi))[0]
        else:
            ids_hi = np.array([], np.int64)
            ids_int = np.array([], np.int64)
        loc.append((ids_lo, ids_hi, ids_int))

    maxint = max(len(t[2]) for t in loc)
    Lc = 2 * Wp + maxint
    T2 = (Lc + P - 1) // P
    Lc = T2 * P

    coarse_local = np.full((NCORE, Lc), -1, np.int64)
    coarse_g2l = np.full(n2, -1, np.int64)
    for c in range(NCORE):
        ids_lo, ids_hi, ids_int = loc[c]
        coarse_local[c, :len(ids_lo)] = ids_lo
        coarse_local[c, Wp:Wp + len(ids_hi)] = ids_hi
        coarse_local[c, 2 * Wp:2 * Wp + len(ids_int)] = ids_int
        coarse_g2l[ids_lo] = np.arange(len(ids_lo))
        coarse_g2l[ids_hi] = Wp + np.arange(len(ids_hi))
        coarse_g2l[ids_int] = 2 * Wp + np.arange(len(ids_int))

    fslab = slab[parent]
    fine_ids = [np.where(fslab == c)[0] for c in range(NCORE)]
    maxfine = max(len(a) for a in fine_ids)
    T1 = (maxfine + P - 1) // P
    Lf = T1 * P
    fine_local = np.full((NCORE, Lf), -1, np.int64)
    fine_g2l = np.full(N1, -1, np.int64)
    for c in range(NCORE):
        a = fine_ids[c]
        fine_local[c, :len(a)] = a
        fine_g2l[a] = np.arange(len(a))

    fxmax = int(fc[:, 0].max())
    DX, DY, DZ = fxmax + 3, 130, 130
    dense_id = ((fc[:, 0] + 1) * DY + (fc[:, 1] + 1)) * DZ + (fc[:, 2] + 1)
    Vdf = DX * DY * DZ

    G1 = np.zeros((NCORE, Lf, 3), np.int64)
    for c in range(NCORE):
        ids = fine_local[c]
        m = ids >= 0
        base = np.zeros((Lf, 3), np.int64)
        base[m] = fc[ids[m]]
        for gx, dx in enumerate((-1, 0, 1)):
            did = ((base[:, 0] + 1 + dx) * DY +
                   (base[:, 1] + 1)) * DZ + (base[:, 2] + 1 - 1)
            did[~m] = 0
            G1[c, :, gx] = did
    assert G1.max() < Vdf and G1.min() >= 0

    Z2_rows = Lf * 8 + 2
    Z2_zero = Lf * 8
    Z2_bias = Lf * 8 + 1
    G2 = np.full((NCORE, Lc, K8 + 1), Z2_zero, np.int64)
    G2[:, :, K8] = Z2_bias
    for k in range(K8):
        oi = map2_out[k]; ii = map2_in[k]
        valid = oi != n2
        o = oi[valid]; i_ = ii[valid]
        c_of = slab[o]
        for c in range(NCORE):
            m = c_of == c
            G2[c, coarse_g2l[o[m]], k] = fine_g2l[i_[m]] * 8 + k
    assert G2.max() < Z2_rows

    halo_base = Lc
    Z3_rows = (Lc + NCORE * 2 * Wp) * K27 + 32
    Z3_zero = (Lc + NCORE * 2 * Wp) * K27
    Z3_bias = Z3_zero + 1
    G3 = np.full((NCORE, Lc, K27 + 1), Z3_zero, np.int64)
    G3[:, :, K27] = Z3_bias
    for k in range(K27):
        oi = map3_out[k]; ii = map3_in[k]
        valid = oi != n2
        o = oi[valid]; i_ = ii[valid]
        c_of = slab[o]
        i_slab = slab[i_]
        for c in range(NCORE):
            m = c_of == c
            om = o[m]; im = i_[m]
            own = i_slab[m] == c
            G3[c, coarse_g2l[om[own]], k] = coarse_g2l[im[own]] * K27 + k
            d = i_slab[m][~own]
            hp = coarse_g2l[im[~own]]
            assert (hp < 2 * Wp).all()
            G3[c, coarse_g2l[om[~own]], k] = (halo_base + d * 2 * Wp + hp) * K27 + k
    assert G3.max() < Z3_rows

    W1 = np.asarray(inputs['W1'], np.float32)
    W2 = np.asarray(inputs['W2'], np.float32)
    W3 = np.asarray(inputs['W3'], np.float32)
    W1all = np.zeros((216, C), np.float32)
    for dxi in range(3):
        for dyi in range(3):
            for dzi in range(3):
                k = dxi * 9 + dyi * 3 + dzi
                r = dxi * 72 + dzi * 24 + dyi * 8
                W1all[r:r + 8, :] = W1[k]
    W2all = np.zeros((C, 8 * C), np.float32)
    for j in range(K8):
        W2all[:, j * C:(j + 1) * C] = W2[j]
    W3all = np.zeros((C, K27 * 8), np.float32)
    for k in range(K27):
        W3all[:, k * 8:(k + 1) * 8] = W3[k]

    return dict(
        n2=n2, Wp=Wp, Lc=Lc, Lf=Lf, T1=T1, T2=T2, Vdf=Vdf,
        dense_id=dense_id, coarse_local=coarse_local, fine_local=fine_local,
        G1=G1, G2=G2, G3=G3, W1all=W1all, W2all=W2all, W3all=W3all,
        Z2_rows=Z2_rows, Z3_rows=Z3_rows, Z3_zero=Z3_zero,
        b1=np.asarray(inputs['b1'], np.float32),
        b2=np.asarray(inputs['b2'], np.float32),
        b3=np.asarray(inputs['b3'], np.float32),
    )


# ---------------------------------------------------------------- device
def _build(info, TB1=16, TB2=8, TB3=16):
    from contextlib import ExitStack
    import concourse.bass as bass
    import concourse.mybir as mybir
    from concourse import tile
    dt = mybir.dt

    T1, T2, Wp, Lc, Lf = info['T1'], info['T2'], info['Wp'], info['Lc'], info['Lf']
    Vdf = info['Vdf']
    Z2_rows, Z3_rows, Z3_zero = info['Z2_rows'], info['Z3_rows'], info['Z3_zero']

    nc = bass.Bass()

    fd = nc.declare_dram_parameter("fd", [Vdf, 24], dt.float32, isOutput=False)
    g1d = nc.declare_dram_parameter("g1", [P, T1 * 3], dt.int32, isOutput=False)
    g2d = nc.declare_dram_parameter("g2", [P, T2 * 9], dt.int32, isOutput=False)
    g3d = nc.declare_dram_parameter("g3", [P, T2 * 28], dt.int32, isOutput=False)
    w1ad = nc.declare_dram_parameter("w1a", [128, 64], dt.bfloat16, isOutput=False)
    w1bd = nc.declare_dram_parameter("w1b", [128, 64], dt.bfloat16, isOutput=False)
    w2d = nc.declare_dram_parameter("w2", [64, 512], dt.bfloat16, isOutput=False)
    w3d = nc.declare_dram_parameter("w3", [64, 216], dt.bfloat16, isOutput=False)
    b1d = nc.declare_dram_parameter("b1v", [64, 1], dt.float32, isOutput=False)
    zb2d = nc.declare_dram_parameter("zb2", [2, 64], dt.bfloat16, isOutput=False)
    zb3d = nc.declare_dram_parameter("zb3", [32, 8], dt.bfloat16, isOutput=False)
    yout = nc.declare_dram_parameter("yout", [Lc, 8], dt.float32, isOutput=True)
    import os as _os
    _dbg = _os.environ.get("KDBG", "0") == "1"
    if _dbg:
        dbgZ2 = nc.declare_dram_parameter("dbgZ2", [4096, 64], dt.bfloat16, isOutput=True)
        dbgZ3 = nc.declare_dram_parameter("dbgZ3", [4096, 8], dt.bfloat16, isOutput=True)
        dbgH = nc.declare_dram_parameter("dbgH", [4096, 8], dt.bfloat16, isOutput=True)

    Z2 = nc.dram_tensor("Z2", [Z2_rows, 64], dt.bfloat16)
    Z3tab = nc.dram_tensor("Z3tab", [Z3_rows, 8], dt.bfloat16)
    Z3win = nc.dram_tensor("Z3win", [2 * Wp * K27, 8], dt.bfloat16)
    Z3halo = nc.dram_tensor("Z3halo", [NCORE * 2 * Wp * K27, 8], dt.bfloat16,
                            addr_space="Shared")

    # ---------------- raw prelude: constants + tail rows ----------------
    ctx = ExitStack()
    g1sb = ctx.enter_context(nc.sbuf_tensor([P, T1 * 3], dt.int32))
    g2sb = ctx.enter_context(nc.sbuf_tensor([P, T2 * 9], dt.int32))
    g3sb = ctx.enter_context(nc.sbuf_tensor([P, T2 * 28], dt.int32))
    w1a = ctx.enter_context(nc.sbuf_tensor([128, 64], dt.bfloat16))
    w1b = ctx.enter_context(nc.sbuf_tensor([128, 64], dt.bfloat16))
    w2 = ctx.enter_context(nc.sbuf_tensor([64, 512], dt.bfloat16))
    w3 = ctx.enter_context(nc.sbuf_tensor([64, 216], dt.bfloat16))
    b1sb = ctx.enter_context(nc.sbuf_tensor([64, 1], dt.float32))
    zb2 = ctx.enter_context(nc.sbuf_tensor([2, 64], dt.bfloat16))
    zb3 = ctx.enter_context(nc.sbuf_tensor([32, 8], dt.bfloat16))
    ident = ctx.enter_context(nc.sbuf_tensor([P, P], dt.float32))

    # work buffers (double/triple buffered)
    NIM, NZ2 = 3, 3
    imb = [ctx.enter_context(nc.sbuf_tensor(f"imb{i}", [P, TB1 * 216], dt.float32))
           for i in range(NIM)]
    r1b = [ctx.enter_context(nc.sbuf_tensor(f"r1b{i}", [128, 128], dt.bfloat16)) for i in range(2)]
    r2b = [ctx.enter_context(nc.sbuf_tensor(f"r2b{i}", [128, 128], dt.bfloat16)) for i in range(2)]
    h1Tb = [ctx.enter_context(nc.sbuf_tensor(f"h1Tb{i}", [64, 128], dt.bfloat16)) for i in range(2)]
    z2b = [ctx.enter_context(nc.sbuf_tensor(f"z2b{i}", [128, 512], dt.bfloat16))
           for i in range(NZ2)]
    d2b = [ctx.enter_context(nc.sbuf_tensor(f"d2b{i}", [P, TB2 * 576], dt.bfloat16))
           for i in range(2)]
    redb = [ctx.enter_context(nc.sbuf_tensor(f"redb{i}", [128, 64], dt.float32)) for i in range(2)]
    h2b = [ctx.enter_context(nc.sbuf_tensor(f"h2b{i}", [128, 64], dt.float32)) for i in range(2)]
    h2Tb = [ctx.enter_context(nc.sbuf_tensor(f"h2Tb{i}", [64, 128], dt.bfloat16)) for i in range(2)]
    z3b = [ctx.enter_context(nc.sbuf_tensor(f"z3b{i}", [128, 216], dt.bfloat16))
           for i in range(NZ2)]
    d3b = [ctx.enter_context(nc.sbuf_tensor(f"d3b{i}", [P, TB3 * 224], dt.bfloat16))
           for i in range(2)]
    ob = [ctx.enter_context(nc.sbuf_tensor(f"ob{i}", [128, 8], dt.float32)) for i in range(3)]

    pt1b = [ctx.enter_context(nc.psum_tensor(f"pt1b{i}", [128, 128], dt.float32))
            for i in range(2)]
    pt2b = [ctx.enter_context(nc.psum_tensor(f"pt2b{i}", [128, 128], dt.float32))
            for i in range(2)]
    pyb = [ctx.enter_context(nc.psum_tensor(f"pyb{i}", [64, 128], dt.float32)) for i in range(2)]
    pzb = [ctx.enter_context(nc.psum_tensor(f"pzb{i}", [128, 512], dt.float32)) for i in range(2)]

    sem = {}
    names = ["s_ld", "s_id", "sVr", "sTt", "sVcp", "sTmm", "sAr", "sTz",
             "sVz", "sVh2", "sTt2", "sVhT", "sTz3", "sVz3",
             "sVo", "sh", "scc"]
    names += [f"sGim{i}" for i in range(NIM)]
    names += [f"sS{i}" for i in range(NZ2)]
    names += ["sGd20", "sGd21"]
    names += [f"sS2_{i}" for i in range(NZ2)]
    names += ["sGd30", "sGd31"]
    names += [f"sS3_{i}" for i in range(3)]
    for name in names:
        sem[name] = ctx.enter_context(nc.semaphore(name))

    loads = [(g1sb, g1d), (g2sb, g2d), (g3sb, g3d), (w1a, w1ad), (w1b, w1bd),
             (w2, w2d), (w3, w3d), (b1sb, b1d), (zb2, zb2d), (zb3, zb3d)]
    for dst, src in loads:
        nc.sync.dma_start(out=dst[:], in_=src[:]).then_inc(sem["s_ld"], 16)
    nc.sync.wait_ge(sem["s_ld"], 16 * len(loads))
    nc.sync.dma_start(out=Z2[Lf * 8:Lf * 8 + 2, :], in_=zb2[:]).then_inc(sem["s_ld"], 16)
    nc.sync.dma_start(out=Z3tab[Z3_zero:Z3_zero + 32, :], in_=zb3[:]).then_inc(
        sem["s_ld"], 16)
    nc.gpsimd.memset(ident[:], 0.0).then_inc(sem["s_id"], 1)
    nc.gpsimd.wait_ge(sem["s_id"], 1)
    nc.gpsimd.affine_select(
        out=ident[:], in_=ident[:],
        compare_op=mybir.AluOpType.not_equal, fill=1.0, base=0,
        pattern=[[-1, P]], channel_multiplier=1,
    ).then_inc(sem["s_id"], 1)
    nc.gpsimd.memset(r2b[0][64:128, :], 0.0).then_inc(sem["s_id"], 1)
    nc.gpsimd.memset(r2b[1][64:128, :], 0.0).then_inc(sem["s_id"], 1)
    nwait = 16 * (len(loads) + 2)
    for eng in (nc.sync, nc.gpsimd, nc.tensor, nc.vector, nc.scalar):
        eng.wait_ge(sem["s_ld"], nwait)
        eng.wait_ge(sem["s_id"], 4)

    def W(eng, s, v):
        if v > 0:
            eng.wait_ge(sem[s], v)

    # ================= phase 1 =================
    NB1 = (T1 + TB1 - 1) // TB1
    batch_of = lambda t: t // TB1
    end_tile = lambda b: min((b + 1) * TB1, T1)

    # gpsimd: all gather batches ([128,1]-offset calls; multi-idx broken on HW)
    gim_cnt = [0] * NIM
    gim_at = {}
    for b in range(NB1):
        nt = min(TB1, T1 - b * TB1)
        if b >= NIM:
            W(nc.gpsimd, "sTt", end_tile(b - NIM))
        sl = b % NIM
        for j in range(nt * 3):
            nc.gpsimd.indirect_dma_start(
                out=imb[sl][:, j * 72:(j + 1) * 72], out_offset=None, in_=fd[:],
                in_offset=bass.IndirectOffsetOnAxis(
                    ap=g1sb[:, b * TB1 * 3 + j:b * TB1 * 3 + j + 1], axis=0),
            ).then_inc(sem[f"sGim{sl}"], 16)
            gim_cnt[sl] += 16
        gim_at[b] = gim_cnt[sl]

    for t in range(T1):
        b = batch_of(t)
        tt = t - b * TB1
        s_ap = imb[b % NIM][:, tt * 216:(tt + 1) * 216]
        i2 = t % 2

        # --- tensor engine ---
        if tt == 0:
            W(nc.tensor, f"sGim{b % NIM}", gim_at[b])
        W(nc.tensor, "sVcp", 2 * (t - 2) + 2)
        nc.tensor.transpose(out=pt1b[i2][:], in_=s_ap[:, 0:128], identity=ident[:])
        nc.tensor.transpose(out=pt2b[i2][:88, :], in_=s_ap[:, 128:216],
                            identity=ident[:]).then_inc(sem["sTt"], 1)
        W(nc.tensor, "sVcp", 2 * t + 2)
        W(nc.tensor, "sAr", t - 1)
        nc.tensor.matmul(out=pyb[i2][:], lhsT=w1a[:], rhs=r1b[i2][:],
                         start=True, stop=False)
        nc.tensor.matmul(out=pyb[i2][:], lhsT=w1b[:], rhs=r2b[i2][:],
                         start=False, stop=True).then_inc(sem["sTmm"], 1)
        W(nc.tensor, "sAr", t + 1)
        W(nc.tensor, "sVz", t - 1)
        nc.tensor.matmul(out=pzb[i2][:], lhsT=h1Tb[i2][:], rhs=w2[:],
                         start=True, stop=True).then_inc(sem["sTz"], 1)

        # --- vector engine ---
        W(nc.vector, "sTt", t + 1)
        W(nc.vector, "sTmm", t - 1)
        nc.vector.tensor_copy(out=r1b[i2][:], in_=pt1b[i2][:])
        nc.vector.tensor_copy(out=r2b[i2][:88, :], in_=pt2b[i2][:88, :]).then_inc(
            sem["sVcp"], 2)
        if t >= 1:
            W(nc.vector, "sTz", t)
            W(nc.vector, f"sS{(t - 1) % NZ2}", 16 * ((t - 1) // NZ2))
            nc.vector.tensor_copy(out=z2b[(t - 1) % NZ2][:],
                                  in_=pzb[(t - 1) % 2][:]).then_inc(sem["sVz"], 1)

        # --- scalar engine ---
        W(nc.scalar, "sTmm", t + 1)
        W(nc.scalar, "sTz", t - 1)
        nc.scalar.activation(out=h1Tb[i2][:], in_=pyb[i2][:],
                             func=mybir.ActivationFunctionType.Relu,
                             bias=b1sb[:, 0:1]).then_inc(sem["sAr"], 1)

        # --- sync engine: write z2 of tile t-1 ---
        if t >= 1:
            W(nc.sync, "sVz", t)
            nc.sync.dma_start(
                out=Z2[(t - 1) * 1024:t * 1024, :].rearrange(
                    "(p j) c -> p (j c)", p=128),
                in_=z2b[(t - 1) % NZ2][:]).then_inc(sem[f"sS{(t - 1) % NZ2}"], 16)
    # tail
    W(nc.vector, "sTz", T1)
    W(nc.vector, f"sS{(T1 - 1) % NZ2}", 16 * ((T1 - 1) // NZ2))
    nc.vector.tensor_copy(out=z2b[(T1 - 1) % NZ2][:],
                          in_=pzb[(T1 - 1) % 2][:]).then_inc(sem["sVz"], 1)
    W(nc.sync, "sVz", T1)
    nc.sync.dma_start(
        out=Z2[(T1 - 1) * 1024:T1 * 1024, :].rearrange("(p j) c -> p (j c)", p=128),
        in_=z2b[(T1 - 1) % NZ2][:]).then_inc(sem[f"sS{(T1 - 1) % NZ2}"], 16)
    # phase-1 completion barrier: all engines wait for all Z2 writes
    for eng in (nc.sync, nc.gpsimd, nc.tensor, nc.vector, nc.scalar):
        for k in range(NZ2):
            cnt = len([t for t in range(T1) if t % NZ2 == k])
            eng.wait_ge(sem[f"sS{k}"], 16 * cnt)

    # ================= phase 2 =================
    NB2 = (T2 + TB2 - 1) // TB2
    end_tile2 = lambda b: min((b + 1) * TB2, T2)
    gd2_cnt = [0, 0]
    gd2_at = {}
    for b in range(NB2):
        nt = min(TB2, T2 - b * TB2)
        if b >= 2:
            W(nc.gpsimd, "sVh2", end_tile2(b - 2))
        sl = b % 2
        for j in range(nt * 9):
            nc.gpsimd.indirect_dma_start(
                out=d2b[sl][:, j * 64:(j + 1) * 64], out_offset=None, in_=Z2[:],
                in_offset=bass.IndirectOffsetOnAxis(
                    ap=g2sb[:, b * TB2 * 9 + j:b * TB2 * 9 + j + 1], axis=0),
            ).then_inc(sem[f"sGd2{sl}"], 16)
            gd2_cnt[sl] += 16
        gd2_at[b] = gd2_cnt[sl]

    for t in range(T2):
        b = t // TB2
        tt = t - b * TB2
        s_ap = d2b[b % 2][:, tt * 576:(tt + 1) * 576]
        i2 = t % 2

        # --- vector: reduce + relu ---
        if tt == 0:
            W(nc.vector, f"sGd2{b % 2}", gd2_at[b])
        W(nc.vector, "sTt2", t - 1)
        nc.vector.tensor_reduce(
            out=redb[i2][:], in_=s_ap.rearrange("p (j c) -> p c j", j=9),
            axis=mybir.AxisListType.X, op=mybir.AluOpType.add).then_inc(
            sem["sVr"], 1)
        W(nc.vector, "sVr", t + 1)
        nc.vector.tensor_scalar_max(out=h2b[i2][:], in0=redb[i2][:],
                                    scalar1=0.0).then_inc(sem["sVh2"], 1)
        if t >= 1:
            W(nc.vector, "sTt2", t)
            W(nc.vector, "sTz3", t - 2)
            nc.vector.tensor_copy(out=h2Tb[(t - 1) % 2][:],
                                  in_=pt1b[(t - 1) % 2][:64, :]).then_inc(
                sem["sVhT"], 1)
            W(nc.vector, "sTz3", t)
            W(nc.vector, f"sS2_{(t - 1) % NZ2}", 16 * ((t - 1) // NZ2))
            nc.vector.tensor_copy(out=z3b[(t - 1) % NZ2][:],
                                  in_=pzb[(t - 1) % 2][:, 0:216]).then_inc(
                sem["sVz3"], 1)

        # --- tensor: transpose(t), then matmul(t-1) ---
        W(nc.tensor, "sVh2", t + 1)
        W(nc.tensor, "sVhT", t - 1)
        nc.tensor.transpose(out=pt1b[i2][:64, :], in_=h2b[i2][:],
                            identity=ident[:]).then_inc(sem["sTt2"], 1)
        if t >= 1:
            W(nc.tensor, "sVhT", t)
            W(nc.tensor, "sVz3", t - 2)
            nc.tensor.matmul(out=pzb[(t - 1) % 2][:, 0:216],
                             lhsT=h2Tb[(t - 1) % 2][:], rhs=w3[:],
                             start=True, stop=True).then_inc(sem["sTz3"], 1)

        # --- sync: write z3 of t-2 ---
        if t >= 2:
            W(nc.sync, "sVz3", t - 1)
            nc.sync.dma_start(
                out=Z3tab[(t - 2) * 3456:(t - 1) * 3456, :].rearrange(
                    "(p k) c -> p (k c)", p=128),
                in_=z3b[(t - 2) % NZ2][:]).then_inc(sem[f"sS2_{(t - 2) % NZ2}"], 16)
    # tail of phase 2
    t = T2
    W(nc.vector, "sTt2", t)
    nc.vector.tensor_copy(out=h2Tb[(t - 1) % 2][:],
                          in_=pt1b[(t - 1) % 2][:64, :]).then_inc(sem["sVhT"], 1)
    W(nc.tensor, "sVhT", t)
    nc.tensor.matmul(out=pzb[(t - 1) % 2][:, 0:216], lhsT=h2Tb[(t - 1) % 2][:],
                     rhs=w3[:], start=True, stop=True).then_inc(sem["sTz3"], 1)
    W(nc.vector, "sTz3", t)
    W(nc.vector, f"sS2_{(t - 1) % NZ2}", 16 * ((t - 1) // NZ2))
    nc.vector.tensor_copy(out=z3b[(t - 1) % NZ2][:],
                          in_=pzb[(t - 1) % 2][:, 0:216]).then_inc(sem["sVz3"], 1)
    for tl in (T2 - 1, T2):
        W(nc.sync, "sVz3", tl)
        nc.sync.dma_start(
            out=Z3tab[(tl - 1) * 3456:tl * 3456, :].rearrange(
                "(p k) c -> p (k c)", p=128),
            in_=z3b[(tl - 1) % NZ2][:]).then_inc(sem[f"sS2_{(tl - 1) % NZ2}"], 16)
    for eng in (nc.sync, nc.gpsimd, nc.tensor, nc.vector, nc.scalar):
        for k in range(NZ2):
            cnt = len([t for t in range(T2) if t % NZ2 == k])
            eng.wait_ge(sem[f"sS2_{k}"], 16 * cnt)

    # ================= halo exchange =================
    nc.sync.dma_start(out=Z3win[:], in_=Z3tab[0:2 * Wp * K27, :]).then_inc(
        sem["sh"], 16)
    nc.gpsimd.wait_ge(sem["sh"], 16)
    nc.gpsimd.collective_compute(
        "AllGather", mybir.AluOpType.bypass,
        replica_groups=[list(range(NCORE))],
        ins=[Z3win[:]], outs=[Z3halo[:]],
    ).then_inc(sem["scc"], 1)
    nc.sync.wait_ge(sem["scc"], 1)
    nc.sync.dma_start(
        out=Z3tab[Lc * K27:Lc * K27 + NCORE * 2 * Wp * K27, :],
        in_=Z3halo[:]).then_inc(sem["sh"], 16)
    nc.gpsimd.wait_ge(sem["sh"], 32)
    nc.vector.wait_ge(sem["sh"], 32)

    # ================= phase 3 =================
    NB3 = (T2 + TB3 - 1) // TB3
    end_tile3 = lambda b: min((b + 1) * TB3, T2)
    gd3_cnt = [0, 0]
    gd3_at = {}
    for b in range(NB3):
        nt = min(TB3, T2 - b * TB3)
        if b >= 2:
            W(nc.gpsimd, "sVo", end_tile3(b - 2))
        sl = b % 2
        for j in range(nt * 28):
            nc.gpsimd.indirect_dma_start(
                out=d3b[sl][:, j * 8:(j + 1) * 8], out_offset=None, in_=Z3tab[:],
                in_offset=bass.IndirectOffsetOnAxis(
                    ap=g3sb[:, b * TB3 * 28 + j:b * TB3 * 28 + j + 1], axis=0),
            ).then_inc(sem[f"sGd3{sl}"], 16)
            gd3_cnt[sl] += 16
        gd3_at[b] = gd3_cnt[sl]
    for t in range(T2):
        b = t // TB3
        tt = t - b * TB3
        s_ap = d3b[b % 2][:, tt * 224:(tt + 1) * 224]
        if tt == 0:
            W(nc.vector, f"sGd3{b % 2}", gd3_at[b])
        W(nc.vector, f"sS3_{t % 3}", 16 * (t // 3))
        nc.vector.tensor_reduce(
            out=ob[t % 3][:], in_=s_ap.rearrange("p (j c) -> p c j", j=28),
            axis=mybir.AxisListType.X, op=mybir.AluOpType.add).then_inc(
            sem["sVo"], 1)
        W(nc.sync, "sVo", t + 1)
        nc.sync.dma_start(out=yout[t * 128:(t + 1) * 128, :],
                          in_=ob[t % 3][:]).then_inc(sem[f"sS3_{t % 3}"], 16)
    for k in range(3):
        cnt = len([t for t in range(T2) if t % 3 == k])
        nc.sync.wait_ge(sem[f"sS3_{k}"], 16 * cnt)
    if _dbg:
        nc.sync.dma_start(out=dbgZ2[:], in_=Z2[0:4096, :]).then_inc(sem["sh"], 16)
        nc.sync.dma_start(out=dbgZ3[:], in_=Z3tab[0:4096, :]).then_inc(sem["sh"], 16)
        nc.sync.dma_start(out=dbgH[:],
                          in_=Z3tab[Lc * K27:Lc * K27 + 4096, :]).then_inc(sem["sh"], 16)
        nc.sync.wait_ge(sem["sh"], 80)

    ctx.close()
    return nc


# ---------------------------------------------------------------- entry
_GRAPH_CACHE = {}


def _ensure_trace_shim():
    """bass_utils imports antenv.axon_hooks when BASS_TRACE is set; the agent
    image lacks that module. Provide it (with the ctypes NTFF hook if the
    axon boot files are present)."""
    import os, types
    if "antenv.axon_hooks" in sys.modules or not os.environ.get("BASS_TRACE"):
        return
    try:
        import antenv
        mod = types.ModuleType("antenv.axon_hooks")
        state = {"hook": None}
        mod.set_axon_ntff_profile_hook = lambda h: state.__setitem__("hook", h)
        mod.get_axon_ntff_profile_hook = lambda: state["hook"]
        sys.modules["antenv.axon_hooks"] = mod
        antenv.axon_hooks = mod
        sys.path.insert(0, "/root/.axon_site")
        from trn_agent_boot.trn_boot import _ntff_profile_via_ctypes
        mod.set_axon_ntff_profile_hook(
            _ntff_profile_via_ctypes("/opt/axon/libaxon_pjrt.so"))
    except Exception:
        pass


def kernel(feats, W1, b1, W2, b2, W3, b3,
           map1_in, map1_out, map2_in, map2_out, map3_in, map3_out, n2):
    import ml_dtypes
    _ensure_trace_shim()
    from concourse.bass_utils import run_bass_kernel_spmd
    bf16 = ml_dtypes.bfloat16

    inputs = dict(feats=np.asarray(feats), W1=W1, b1=b1, W2=W2, b2=b2,
                  W3=W3, b3=b3, map2_in=np.asarray(map2_in),
                  map2_out=np.asarray(map2_out), map3_in=np.asarray(map3_in),
                  map3_out=np.asarray(map3_out), n2=int(n2))
    info = _prepare(inputs)
    T1, T2, Lc, Lf = info['T1'], info['T2'], info['Lc'], info['Lf']

    feats_f = np.asarray(feats, np.float32)
    DZv = 130
    fd0 = np.zeros((info['Vdf'], 8), np.float32)
    fd0[info['dense_id']] = feats_f
    fdense = np.zeros((info['Vdf'], 24), np.float32)
    fdense[DZv:, 0:8] = fd0[:-DZv]
    fdense[:, 8:16] = fd0
    fdense[:-DZv, 16:24] = fd0[DZv:]

    w1a = np.zeros((128, 64), np.float32)
    w1b = np.zeros((128, 64), np.float32)
    w1a[:] = info['W1all'][0:128]
    w1b[:88] = info['W1all'][128:216]
    zb2 = np.zeros((2, 64), bf16)
    zb2[1] = info['b2'].astype(bf16)
    zb3 = np.zeros((32, 8), bf16)
    zb3[1] = info['b3'].astype(bf16)

    shared = dict(
        fd=fdense,
        w1a=w1a.astype(bf16), w1b=w1b.astype(bf16),
        w2=info['W2all'].astype(bf16), w3=info['W3all'].astype(bf16),
        b1v=info['b1'].reshape(64, 1).astype(np.float32),
        zb2=zb2, zb3=zb3,
    )
    in_maps = []
    for c in range(NCORE):
        m = dict(shared)
        m['g1'] = np.ascontiguousarray(
            info['G1'][c].reshape(T1, P, 3).transpose(1, 0, 2)
            .reshape(P, T1 * 3).astype(np.int32))
        m['g2'] = np.ascontiguousarray(
            info['G2'][c].reshape(T2, P, 9).transpose(1, 0, 2)
            .reshape(P, T2 * 9).astype(np.int32))
        m['g3'] = np.ascontiguousarray(
            info['G3'][c].reshape(T2, P, 28).transpose(1, 0, 2)
            .reshape(P, T2 * 28).astype(np.int32))
        in_maps.append(m)

    key = (T1, T2, info['Wp'], info['Vdf'])
    if key not in _GRAPH_CACHE:
        _GRAPH_CACHE[key] = _build(info)
    nc = _GRAPH_CACHE[key]

    res = run_bass_kernel_spmd(nc, in_maps, list(range(NCORE)))
    kernel.last_hw_ns = res.exec_time_ns or 0

    out = np.zeros((info['n2'], 8), np.float32)
    for c in range(NCORE):
        ids = info['coarse_local'][c]
        mm = ids >= 0
        yc = np.asarray(res.results[c]['yout'], np.float32)
        out[ids[mm]] = yc[mm]
    return out
